# revision 1
# baseline (speedup 1.0000x reference)
"""GATv2 8-hop GNN on 8 Trainium2 NeuronCores (Bass/Tile) — v2.

Strategy (receiver-sharded, degree-tiered, batched DMA):
  - Nodes partitioned across 8 cores balanced by in-degree; each core owns
    its nodes' full in-edge lists.  Nodes grouped into degree tiers (padded
    to d in TIERS), packed into tiles of 128 nodes x d edge slots.
  - Per hop, per core: phase A computes kn|q = h @ [Wk|Wq] per tile into an
    SBUF staging table, ONE big DMA writes the fp16 kn table to DRAM,
    AllGather replicates it.  Phase C per tile:
      ONE batched indirect DMA gathers all d*128 kn rows (idx [128, d])
      ONE DMA loads the tile's edge slab [H, d*128] (4KB descriptors)
      per chunk of 8 slots: z = ke + kn[snd] + (q+b)  (3 matmuls/slot, PSUM)
        u = Lrelu(z) on ACT engine; logits = sum(u * 0.6a) (DVE)
        ex = exp(logits + logmask); den += sum(ex)
        exz = z * ex (from PSUM); pooled += id @ exz (PE)
      h' = relu((pooled/den - qb) * valid)
  - Output: per-core column sums of h; host reduces across cores.
"""
import os
import sys

sys.path.insert(0, "/opt/trn_rl_repo")

import numpy as np
from contextlib import ExitStack

import concourse.bass as bass
import concourse.mybir as mybir
import concourse.tile as tile
from concourse.bass import AP, ds
from concourse.masks import make_identity

P = 128
H = 64
HOPS = int(os.environ.get("GAT_HOPS", "8"))
NCORES = 8
CH = 8  # slots per PSUM chunk
TIERS = [8, 12, 16, 20, 24, 28, 32, 40, 48, 64, 96, 128]
UNROLL_OF_D = {8: 4, 12: 4, 16: 4, 20: 4, 24: 4, 28: 4, 32: 2, 40: 2,
               48: 2, 64: 1, 96: 1, 128: 1}
EPS = 1e-30
NEG = -30000.0
F16 = mybir.dt.float16
F32 = mybir.dt.float32
I32 = mybir.dt.int32


# ----------------------------------------------------------------- host prep
def preprocess(node_feats, edge_feats, senders, receivers, Wq, Wk, We, b, att):
    """Shard + tile the graph.  Returns (meta, in_maps)."""
    N = node_feats.shape[0]
    E = edge_feats.shape[0]
    deg = np.bincount(receivers, minlength=N)
    assert deg.max() <= TIERS[-1], f"max degree {deg.max()} > {TIERS[-1]}"

    cum = np.cumsum(deg)
    bounds = [0] + [int(np.searchsorted(cum, E * (c + 1) / NCORES))
                    for c in range(NCORES - 1)] + [N]

    tier_arr = np.asarray(TIERS)
    tier_of = np.searchsorted(tier_arr, np.maximum(deg, 1))
    core_tier_nodes = []
    for c in range(NCORES):
        lo, hi = bounds[c], bounds[c + 1]
        nodes = np.arange(lo, hi)
        t_of = tier_of[lo:hi]
        core_tier_nodes.append([nodes[t_of == ti] for ti in range(len(TIERS))])

    T_raw = [max(int(np.ceil(len(core_tier_nodes[c][ti]) / P))
                 for c in range(NCORES)) for ti in range(len(TIERS))]
    T_tier = []
    U_tier = []
    for ti in range(len(TIERS)):
        T = T_raw[ti]
        if not T:
            T_tier.append(0)
            U_tier.append(1)
            continue
        d = TIERS[ti]
        cap = 4 if d <= 32 else (2 if d <= 48 else 1)
        best = None
        for u in (1, 2, 4, 8):
            if u > cap:
                continue
            groups = -(-T // u)
            # barrier ~8us per group; wasted pad tile ~ d*1.04us of gathers
            cost = groups * 8000 + (groups * u - T) * d * 1040
            if best is None or cost < best[0]:
                best = (cost, u, groups * u)
        T_tier.append(best[2])
        U_tier.append(best[1])
    used = [ti for ti in range(len(TIERS)) if T_tier[ti] > 0]
    tiers_d = [TIERS[ti] for ti in used]
    tiers_T = [T_tier[ti] for ti in used]
    tiers_U = [U_tier[ti] for ti in used]
    Ttot = sum(tiers_T)
    n_slab = Ttot * P

    # slab position of each node (per core): index i -> tile i//P, part i%P
    posi = np.zeros(N, np.int64)
    core_of = np.zeros(N, np.int64)
    slab_node = np.full((NCORES, n_slab), -1, np.int64)
    for c in range(NCORES):
        off = 0
        for j, ti in enumerate(used):
            nodes = core_tier_nodes[c][ti]
            posi[nodes] = off + np.arange(len(nodes))
            core_of[nodes] = c
            slab_node[c, off:off + len(nodes)] = nodes
            off += tiers_T[j] * P
    assert off == n_slab
    # kn-table row index for the gather: rows laid out [core, part, tile]
    rowidx = ((core_of * P + (posi % P)) * Ttot + posi // P).astype(np.int64)

    order = np.argsort(receivers, kind="stable")
    starts = np.zeros(N + 1, np.int64)
    np.cumsum(deg, out=starts[1:])

    ef16 = np.concatenate([np.asarray(edge_feats).astype(np.float16),
                           np.zeros((1, H), np.float16)], axis=0)
    snd_row = rowidx[senders]

    in_maps = []
    for c in range(NCORES):
        m = {}
        off = 0
        for j, ti in enumerate(used):
            d = tiers_d[j]
            Tt = tiers_T[j]
            rows = slab_node[c, off:off + Tt * P]
            eids = np.full((Tt * P, d), -1, np.int64)
            real = rows >= 0
            idxr = np.nonzero(real)[0]
            if len(idxr):
                rs = starts[rows[idxr]]
                dd = deg[rows[idxr]]
                ar = np.arange(d)
                sel = ar[None, :] < dd[:, None]
                flat = rs[:, None] + ar[None, :]
                tmp = np.full((len(idxr), d), -1, np.int64)
                tmp[sel] = order[flat[sel]]
                eids[idxr] = tmp
            pad = eids < 0
            e_safe = np.where(pad, E, eids)
            slab = (ef16[e_safe].reshape(Tt, P, d, H)
                    .transpose(0, 3, 2, 1).reshape(Tt, H, d * P).copy())
            m[f"edge{j}"] = slab
            idx = np.where(pad, 0, snd_row[np.maximum(eids, 0)]).astype(np.int32)
            m[f"idx{j}"] = idx.reshape(Tt, P, d).copy()
            m[f"lgm{j}"] = np.where(pad, NEG, 0.0).astype(np.float16) \
                             .reshape(Tt, P, d).copy()
            off += Tt * P
        rows = slab_node[c]
        h0 = np.zeros((n_slab, H), np.float32)
        rr = rows >= 0
        h0[rr] = np.asarray(node_feats)[rows[rr]].astype(np.float32)
        m["h0"] = h0.reshape(Ttot, P, H).copy()
        vv = rr.copy()
        vv[rr] &= deg[rows[rr]] > 0
        m["valid"] = vv.astype(np.float32).reshape(Ttot, P).T.copy()
        m["wkq"] = np.concatenate(
            [np.asarray(Wk)[:HOPS], np.asarray(Wq)[:HOPS]], axis=2) \
            .astype(np.float16)
        m["we"] = np.asarray(We)[:HOPS].astype(np.float16)
        m["brep"] = np.broadcast_to(
            np.asarray(b)[:HOPS].astype(np.float16)[:, None, :],
            (HOPS, P, H)).copy()
        m["arep"] = np.broadcast_to(
            np.asarray(att)[:HOPS].astype(np.float16)[:, None, :],
            (HOPS, P, H)).copy()
        in_maps.append(m)

    meta = dict(tiers_d=tiers_d, tiers_T=tiers_T, tiers_U=tiers_U,
                Ttot=Ttot, n_slab=n_slab,
                slab_node=slab_node)
    return meta, in_maps


# -------------------------------------------------------------- device build
def build_program(tc, t_in, t_out, meta):
    """Emit the program into TileContext tc.  t_in: dict name->AP (DRAM)."""
    nc = tc.nc
    tiers_d = meta["tiers_d"]
    tiers_T = meta["tiers_T"]
    Ttot = meta["Ttot"]

    kn_own = nc.dram_tensor("kn_own", [P, Ttot * H], F16, kind="Internal")
    kn_full = nc.dram_tensor("kn_full", [NCORES * P, Ttot * H], F16,
                             kind="Internal", addr_space="Shared")
    kn_view = kn_full.ap().rearrange("(c p) (t f) -> (c p t) f", p=P, t=Ttot)

    SKIP_A = os.environ.get("GAT_SKIP_A")
    SKIP_AG = os.environ.get("GAT_SKIP_AG")
    SKIP_C = os.environ.get("GAT_SKIP_C")
    SKIP_GATHER = os.environ.get("GAT_SKIP_GATHER")

    with ExitStack() as ctx:
        const = ctx.enter_context(tc.tile_pool(name="const", bufs=1))

        id16 = const.tile([P, P], F16)
        make_identity(nc, id16[:])

        wkq_sb = const.tile([H, HOPS, 2 * H], F16)
        nc.sync.dma_start(wkq_sb[:], t_in["wkq"].rearrange("i k f -> k i f"))
        we_sb = const.tile([H, HOPS, H], F16)
        nc.sync.dma_start(we_sb[:], t_in["we"].rearrange("i k f -> k i f"))
        brep_sb = const.tile([P, HOPS, H], F16)
        nc.sync.dma_start(brep_sb[:], t_in["brep"].rearrange("i p f -> p i f"))
        arep_sb = const.tile([P, HOPS, H], F16)
        nc.sync.dma_start(arep_sb[:], t_in["arep"].rearrange("i p f -> p i f"))
        valid_sb = const.tile([P, Ttot], F32)
        nc.sync.dma_start(valid_sb[:], t_in["valid"])

        h_own = const.tile([P, Ttot, H], F32)
        nc.sync.dma_start(h_own[:], t_in["h0"].rearrange("t p f -> p t f"))
        qb_own = const.tile([P, Ttot, H], F16)
        kn_stage = const.tile([P, Ttot, H], F16)

        for hop in range(HOPS):
            # ---- phase A: kn|q for own nodes into SBUF staging
            with tc.tile_pool(name=f"psA{hop}", bufs=2, space="PSUM") as psA, \
                 tc.tile_pool(name=f"sbA{hop}", bufs=3) as sbA:
                AU = 2

                def bodyA(t, ui):
                    h_t = h_own[:, ds(t, 1), :].rearrange("p o f -> p (o f)")
                    h_stage = sbA.tile([P, H], F16, tag="hst%d" % ui)
                    nc.vector.tensor_copy(h_stage[:], h_t)
                    trp = psA.tile([H, P], F16, tag="trp%d" % ui)
                    nc.tensor.transpose(trp[:], h_stage[:], id16[:])
                    hT16 = sbA.tile([H, P], F16, tag="hT%d" % ui)
                    nc.scalar.copy(hT16[:], trp[:])
                    knq = psA.tile([P, 2 * H], F32, tag="knq%d" % ui)
                    nc.tensor.matmul(knq[:], lhsT=hT16[:],
                                     rhs=wkq_sb[:, hop, :], start=True,
                                     stop=True)
                    kn_t = kn_stage[:, ds(t, 1), :].rearrange("p o f -> p (o f)")
                    nc.scalar.copy(kn_t, knq[:, 0:H])
                    qb_t = qb_own[:, ds(t, 1), :].rearrange("p o f -> p (o f)")
                    nc.vector.tensor_tensor(out=qb_t, in0=knq[:, H:2 * H],
                                            in1=brep_sb[:, hop, :],
                                            op=mybir.AluOpType.add)

                if not SKIP_A:
                    main = Ttot - (Ttot % AU)
                    if main:
                        with tc.For_i(0, main, AU) as t0:
                            for ui in range(AU):
                                bodyA(t0 + ui, ui)
                    for i, tr in enumerate(range(main, Ttot)):
                        bodyA(tr, i % AU)
                    nc.sync.dma_start(
                        kn_own.ap(),
                        kn_stage[:].rearrange("p t f -> p (t f)"))
                else:
                    nc.vector.memset(qb_own[:], 0)

            # ---- AllGather kn table
            if not SKIP_AG:
                nc.gpsimd.collective_compute(
                    "AllGather", mybir.AluOpType.bypass,
                    replica_groups=[list(range(NCORES))],
                    ins=[kn_own.ap()], outs=[kn_full.ap()])

            # ---- phase C: edge processing, one For_i per tier
            tile_base = 0
            for j, d in enumerate(tiers_d):
                Tt = tiers_T[j]
                chunks = [CH] * (d // CH) + ([d % CH] if d % CH else [])
                U = meta["tiers_U"][j]
                qb_tier = qb_own[:, tile_base:tile_base + Tt, :]
                h_tier = h_own[:, tile_base:tile_base + Tt, :]
                valid_tier = valid_sb[:, tile_base:tile_base + Tt]
                with tc.tile_pool(name=f"psC{hop}_{j}", bufs=4,
                                  space="PSUM") as psC, \
                     tc.tile_pool(name=f"psP{hop}_{j}", bufs=2,
                                  space="PSUM") as psP, \
                     tc.tile_pool(name=f"sbC{hop}_{j}", bufs=2) as sbC:

                    def bodyC_z(t, ui, d=d, j=j, chunks=chunks,
                                qb_tier=qb_tier, psC=psC, sbC=sbC):
                        idx_sb = sbC.tile([P, d], I32, tag="idx%d" % ui)
                        nc.sync.dma_start(
                            idx_sb[:],
                            t_in[f"idx{j}"][ds(t, 1)]
                            .rearrange("o p d -> p (o d)"))
                        lgm_sb = sbC.tile([P, d], F16, tag="lgm%d" % ui)
                        nc.sync.dma_start(
                            lgm_sb[:],
                            t_in[f"lgm{j}"][ds(t, 1)]
                            .rearrange("o p d -> p (o d)"))
                        knr = sbC.tile([P, d, H], F16, tag="knr%d" % ui)
                        if SKIP_GATHER:
                            nc.vector.memset(knr[:], 0)
                        else:
                            for s0 in range(d):
                                nc.gpsimd.indirect_dma_start(
                                    out=knr[:, s0, :], out_offset=None,
                                    in_=kn_view,
                                    in_offset=bass.IndirectOffsetOnAxis(
                                        ap=idx_sb[:, s0:s0 + 1], axis=0))
                        edge_sb = sbC.tile([H, d * P], F16, tag="edge%d" % ui)
                        nc.sync.dma_start(
                            edge_sb[:],
                            t_in[f"edge{j}"][ds(t, 1)]
                            .rearrange("o f s -> f (o s)"))
                        qb_t = qb_tier[:, ds(t, 1), :] \
                            .rearrange("p o f -> p (o f)")
                        z16 = sbC.tile([P, d, H], F16, tag="z16%d" % ui)
                        ex_t = sbC.tile([P, d], F16, tag="ex%d" % ui)
                        _a = arep_sb[:, hop, :]
                        for k, c in enumerate(chunks):
                            g0 = k * CH
                            a_b = AP(_a.tensor, _a.offset,
                                     [list(_a.ap[0]), [0, c], list(_a.ap[1])])
                            zps = psC.tile([P, CH * H], F32, tag="z")
                            for s in range(c):
                                sl = slice(s * H, (s + 1) * H)
                                g = g0 + s
                                nc.tensor.matmul(
                                    zps[:, sl],
                                    lhsT=edge_sb[:, g * P:(g + 1) * P],
                                    rhs=we_sb[:, hop, :],
                                    start=True, stop=False)
                                nc.tensor.matmul(zps[:, sl], lhsT=id16[:],
                                                 rhs=knr[:, g, :],
                                                 start=False, stop=False)
                                nc.tensor.matmul(zps[:, sl], lhsT=id16[:],
                                                 rhs=qb_t,
                                                 start=False, stop=True)
                            zc = z16[:, g0:g0 + c, :] \
                                .rearrange("p c h -> p (c h)")
                            nc.scalar.copy(zc, zps[:, 0:c * H])
                            u16 = sbC.tile([P, CH * H], F16, tag="u%d" % ui)
                            uc = u16[:, 0:c * H]
                            nc.vector.scalar_tensor_tensor(
                                out=uc, in0=zc, scalar=0.2,
                                in1=zc, op0=mybir.AluOpType.mult,
                                op1=mybir.AluOpType.max)
                            au = sbC.tile([P, CH, H], F16, tag="au%d" % ui)
                            nc.vector.tensor_tensor(
                                out=au[:, 0:c, :],
                                in0=uc.rearrange("p (c h) -> p c h", c=c),
                                in1=a_b, op=mybir.AluOpType.mult)
                            lgt = sbC.tile([P, CH], F16, tag="lg%d" % ui)
                            with nc.allow_low_precision(reason="O(1) logits"):
                                nc.vector.tensor_reduce(
                                    out=lgt[:, 0:c], in_=au[:, 0:c, :],
                                    axis=mybir.AxisListType.X,
                                    op=mybir.AluOpType.add)
                            lgt2 = sbC.tile([P, CH], F16, tag="lg2%d" % ui)
                            nc.vector.tensor_tensor(
                                out=lgt2[:, 0:c], in0=lgt[:, 0:c],
                                in1=lgm_sb[:, g0:g0 + c],
                                op=mybir.AluOpType.add)
                            nc.scalar.activation(
                                ex_t[:, g0:g0 + c], lgt2[:, 0:c],
                                mybir.ActivationFunctionType.Exp)
                        return z16, ex_t, qb_t

                    def bodyC_pool(t, ui, pooledT, state, d=d, j=j,
                                   chunks=chunks, h_tier=h_tier,
                                   valid_tier=valid_tier, sbC=sbC):
                        z16, ex_t, qb_t = state
                        den = sbC.tile([P, 1], F32, tag="den%d" % ui)
                        with nc.allow_low_precision(reason="den f32"):
                            nc.vector.tensor_reduce(
                                out=den[:], in_=ex_t[:],
                                axis=mybir.AxisListType.X,
                                op=mybir.AluOpType.add)
                        den2 = sbC.tile([P, 1], F32, tag="den2%d" % ui)
                        nc.vector.tensor_scalar(
                            out=den2[:], in0=den[:], scalar1=EPS,
                            scalar2=None, op0=mybir.AluOpType.add)
                        rden = sbC.tile([P, 1], F32, tag="rden%d" % ui)
                        nc.vector.reciprocal(rden[:], den2[:])
                        for k, c in enumerate(chunks):
                            g0 = k * CH
                            exz = sbC.tile([P, CH, H], F16, tag="exz%d" % ui)
                            _m = ex_t[:, g0:g0 + c]
                            m_b = AP(_m.tensor, _m.offset,
                                     [list(_m.ap[0]), list(_m.ap[1]), [0, H]])
                            nc.vector.tensor_tensor(
                                out=exz[:, 0:c, :],
                                in0=z16[:, g0:g0 + c, :],
                                in1=m_b, op=mybir.AluOpType.mult)
                            for s in range(c):
                                nc.tensor.matmul(
                                    pooledT[:, ui * H:(ui + 1) * H],
                                    lhsT=id16[:], rhs=exz[:, s, :],
                                    start=(k == 0 and s == 0),
                                    stop=(k == len(chunks) - 1 and s == c - 1))
                        t1 = sbC.tile([P, H], F32, tag="t1%d" % ui)
                        nc.vector.scalar_tensor_tensor(
                            out=t1[:], in0=pooledT[:, ui * H:(ui + 1) * H],
                            scalar=rden[:], in1=qb_t,
                            op0=mybir.AluOpType.mult,
                            op1=mybir.AluOpType.subtract)
                        h_t = h_tier[:, ds(t, 1), :] \
                            .rearrange("p o f -> p (o f)")
                        nc.vector.tensor_scalar(
                            out=h_t, in0=t1[:],
                            scalar1=valid_tier[:, ds(t, 1)],
                            scalar2=0.0, op0=mybir.AluOpType.mult,
                            op1=mybir.AluOpType.max)

                    if not SKIP_C:
                        with tc.For_i(0, Tt, U) as t0:
                            pooledT = psP.tile([P, U * H], F32, tag="pool")
                            states = [bodyC_z(t0 + ui, ui)
                                      for ui in range(U)]
                            for ui in range(U):
                                bodyC_pool(t0 + ui, ui, pooledT, states[ui])
                tile_base += Tt

        hsum = const.tile([P, H], F32)
        nc.vector.tensor_reduce(out=hsum[:],
                                in_=h_own[:].rearrange("p t f -> p f t"),
                                axis=mybir.AxisListType.X,
                                op=mybir.AluOpType.add)
        nc.sync.dma_start(t_out, hsum[:])


# --------------------------------------------------------------------- entry
def kernel(node_feats, edge_feats, senders, receivers, Wq, Wk, We, b, att):
    from concourse import bacc
    from concourse.bass_utils import run_bass_kernel_spmd

    node_feats = np.asarray(node_feats)
    meta, in_maps = preprocess(node_feats, edge_feats, senders, receivers,
                               Wq, Wk, We, b, att)
    nc = bacc.Bacc("TRN2", target_bir_lowering=False, debug=False,
                   num_devices=NCORES)
    t_in = {}
    for k, v in in_maps[0].items():
        t_in[k] = nc.dram_tensor(k, list(v.shape), mybir.dt.from_np(v.dtype),
                                 kind="ExternalInput").ap()
    t_out = nc.dram_tensor("out", [P, H], F32, kind="ExternalOutput").ap()
    with tile.TileContext(nc) as tc:
        build_program(tc, t_in, t_out, meta)
    nc.compile()
    res = run_bass_kernel_spmd(nc, in_maps, core_ids=list(range(NCORES)))
    global LAST_EXEC_NS
    LAST_EXEC_NS = getattr(res, "exec_time_ns", None)
    total = np.zeros(H, np.float64)
    for r in res.results:
        total += r["out"].astype(np.float64).sum(axis=0)
    return (total / node_feats.shape[0]).astype(np.float32)



# revision 4
# speedup vs baseline: 1.0712x; 1.0712x over previous
"""GATv2 8-hop GNN on 8 Trainium2 NeuronCores (Bass/Tile) — v3.

Strategy (receiver-sharded, per-tile dynamic degree, gather-bound overlap):
  - Nodes partitioned across 8 cores balanced by in-degree; each core owns
    its nodes' full in-edge lists.  Nodes sorted by degree (desc) and packed
    into tiles of 128; each tile's slot count d = max degree in the tile
    (no tier quantization -> ~minimal padded slots = minimal gather
    descriptors, the hard bottleneck at ~1us per 128-row indirect DMA).
  - Per hop, per core: phase A computes kn|q = h @ [Wk|Wq] per tile into an
    SBUF staging table, ONE DMA writes the fp16 kn table to DRAM, AllGather
    replicates it.  Phase C per tile:
      per slot: ONE indirect DMA gathers 128 kn rows (Pool/SWDGE-bound)
      ONE DMA loads the tile's edge slab [H, d*128]
      per chunk of 8 slots: zps = e @ We (PE, PSUM)
        z16 = zps + knr (DVE, v = ke + kn[snd])
        w = z16 + qb; u = lrelu(w); au = u * a; logits = reduce(au) + lgm
        ex = exp(logits) (ACT); exz = z16 * ex; pooled += id @ exz (PE)
      per tile: den = reduce(ex); h' = relu(pooled / den) * valid
  - All compute overlaps the serialized gather stream on Pool.
  - Output: per-core column sums of h; host reduces across cores.
"""
import os
import sys

sys.path.insert(0, "/opt/trn_rl_repo")

import numpy as np
from contextlib import ExitStack

import concourse.bass as bass
import concourse.mybir as mybir
import concourse.tile as tile
from concourse.bass import AP, ds
from concourse.masks import make_identity

P = 128
H = 64
HOPS = int(os.environ.get("GAT_HOPS", "8"))
NCORES = 8
CH = 8  # slots per PSUM chunk
EPS = 1e-30
NEG = -30000.0
F16 = mybir.dt.float16
F32 = mybir.dt.float32
I32 = mybir.dt.int32


def _unroll_for(d, T):
    if d <= 32 and T >= 4:
        return 4
    if d <= 48 and T >= 2:
        return 2
    return 1


# ----------------------------------------------------------------- host prep
def preprocess(node_feats, edge_feats, senders, receivers, Wq, Wk, We, b, att):
    """Shard + tile the graph.  Returns (meta, in_maps)."""
    N = node_feats.shape[0]
    E = edge_feats.shape[0]
    deg = np.bincount(receivers, minlength=N)

    cum = np.cumsum(deg)
    bounds = [0] + [int(np.searchsorted(cum, E * (c + 1) / NCORES))
                    for c in range(NCORES - 1)] + [N]

    # per-core: nodes sorted by degree desc, tiles of 128, d = max deg
    core_nodes = []
    core_tile_d = []
    for c in range(NCORES):
        lo, hi = bounds[c], bounds[c + 1]
        nodes = np.arange(lo, hi)
        order_d = np.argsort(-deg[lo:hi], kind="stable")
        nodes = nodes[order_d]
        core_nodes.append(nodes)
        nt = int(np.ceil(len(nodes) / P))
        dlist = []
        for t in range(nt):
            seg = nodes[t * P:(t + 1) * P]
            dlist.append(max(int(deg[seg].max()), 1))
        core_tile_d.append(dlist)

    # unify tile structure across cores: tile t has d = max over cores
    # (SPMD: all cores run the same program)
    Ttot = max(len(dl) for dl in core_tile_d)
    tile_d = [max((dl[t] if t < len(dl) else 1) for dl in core_tile_d)
              for t in range(Ttot)]
    # runs of equal d
    runs = []  # (d, T)
    for d in tile_d:
        if runs and runs[-1][0] == d:
            runs[-1][1] += 1
        else:
            runs.append([d, 1])
    tiers_d = [r[0] for r in runs]
    tiers_T = [r[1] for r in runs]
    tiers_U = [_unroll_for(d, T) for d, T in runs]
    n_slab = Ttot * P

    # slab position of each node (per core)
    posi = np.zeros(N, np.int64)
    core_of = np.zeros(N, np.int64)
    slab_node = np.full((NCORES, n_slab), -1, np.int64)
    for c in range(NCORES):
        nodes = core_nodes[c]
        posi[nodes] = np.arange(len(nodes))
        core_of[nodes] = c
        slab_node[c, :len(nodes)] = nodes
    # kn-table row index for the gather: rows laid out [core, part, tile]
    rowidx = ((core_of * P + (posi % P)) * Ttot + posi // P).astype(np.int64)

    order = np.argsort(receivers, kind="stable")
    starts = np.zeros(N + 1, np.int64)
    np.cumsum(deg, out=starts[1:])

    ef16 = np.concatenate([np.asarray(edge_feats).astype(np.float16),
                           np.zeros((1, H), np.float16)], axis=0)
    snd_row = rowidx[senders]

    in_maps = []
    for c in range(NCORES):
        m = {}
        t0 = 0
        for j, (dj, Tj) in enumerate(zip(tiers_d, tiers_T)):
            rows = slab_node[c, t0 * P:(t0 + Tj) * P]
            eids = np.full((Tj * P, dj), -1, np.int64)
            real = rows >= 0
            idxr = np.nonzero(real)[0]
            if len(idxr):
                rs = starts[rows[idxr]]
                dd = deg[rows[idxr]]
                ar = np.arange(dj)
                sel = ar[None, :] < dd[:, None]
                flat = rs[:, None] + ar[None, :]
                tmp = np.full((len(idxr), dj), -1, np.int64)
                tmp[sel] = order[flat[sel]]
                eids[idxr] = tmp
            pad = eids < 0
            e_safe = np.where(pad, E, eids)
            slab = (ef16[e_safe].reshape(Tj, P, dj, H)
                    .transpose(0, 3, 2, 1).reshape(Tj, H, dj * P).copy())
            m[f"edge{j}"] = slab
            idx = np.where(pad, 0, snd_row[np.maximum(eids, 0)]).astype(np.int32)
            m[f"idx{j}"] = idx.reshape(Tj, P, dj).copy()
            m[f"lgm{j}"] = np.where(pad, NEG, 0.0).astype(np.float16) \
                             .reshape(Tj, P, dj).copy()
            t0 += Tj
        rows = slab_node[c]
        h0 = np.zeros((n_slab, H), np.float32)
        rr = rows >= 0
        h0[rr] = np.asarray(node_feats)[rows[rr]].astype(np.float32)
        m["h0"] = h0.reshape(Ttot, P, H).copy()
        vv = rr.copy()
        vv[rr] &= deg[rows[rr]] > 0
        m["valid"] = vv.astype(np.float32).reshape(Ttot, P).T.copy()
        m["wkq"] = np.concatenate(
            [np.asarray(Wk)[:HOPS], np.asarray(Wq)[:HOPS]], axis=2) \
            .astype(np.float16)
        m["we"] = np.asarray(We)[:HOPS].astype(np.float16)
        m["brep"] = np.broadcast_to(
            np.asarray(b)[:HOPS].astype(np.float16)[:, None, :],
            (HOPS, P, H)).copy()
        m["arep"] = np.broadcast_to(
            np.asarray(att)[:HOPS].astype(np.float16)[:, None, :],
            (HOPS, P, H)).copy()
        in_maps.append(m)

    meta = dict(tiers_d=tiers_d, tiers_T=tiers_T, tiers_U=tiers_U,
                Ttot=Ttot, n_slab=n_slab,
                slab_node=slab_node)
    return meta, in_maps


# -------------------------------------------------------------- device build
def build_program(tc, t_in, t_out, meta):
    """Emit the program into TileContext tc.  t_in: dict name->AP (DRAM)."""
    nc = tc.nc
    tiers_d = meta["tiers_d"]
    tiers_T = meta["tiers_T"]
    Ttot = meta["Ttot"]

    kn_own = nc.dram_tensor("kn_own", [P, Ttot * H], F16, kind="Internal")
    kn_full = nc.dram_tensor("kn_full", [NCORES * P, Ttot * H], F16,
                             kind="Internal", addr_space="Shared")
    kn_view = kn_full.ap().rearrange("(c p) (t f) -> (c p t) f", p=P, t=Ttot)

    SKIP_A = os.environ.get("GAT_SKIP_A")
    SKIP_AG = os.environ.get("GAT_SKIP_AG")
    SKIP_C = os.environ.get("GAT_SKIP_C")
    SKIP_GATHER = os.environ.get("GAT_SKIP_GATHER")

    with ExitStack() as ctx:
        const = ctx.enter_context(tc.tile_pool(name="const", bufs=1))

        id16 = const.tile([P, P], F16)
        make_identity(nc, id16[:])

        wkq_sb = const.tile([H, HOPS, 2 * H], F16)
        nc.sync.dma_start(wkq_sb[:], t_in["wkq"].rearrange("i k f -> k i f"))
        we_sb = const.tile([H, HOPS, H], F16)
        nc.sync.dma_start(we_sb[:], t_in["we"].rearrange("i k f -> k i f"))
        brep_sb = const.tile([P, HOPS, H], F16)
        nc.sync.dma_start(brep_sb[:], t_in["brep"].rearrange("i p f -> p i f"))
        arep_sb = const.tile([P, HOPS, H], F16)
        nc.sync.dma_start(arep_sb[:], t_in["arep"].rearrange("i p f -> p i f"))
        valid_sb = const.tile([P, Ttot], F32)
        nc.sync.dma_start(valid_sb[:], t_in["valid"])

        h_own = const.tile([P, Ttot, H], F32)
        nc.sync.dma_start(h_own[:], t_in["h0"].rearrange("t p f -> p t f"))
        qb_own = const.tile([P, Ttot, H], F16)
        kn_stage = const.tile([P, Ttot, H], F16)

        for hop in range(HOPS):
            # ---- phase A: kn|q for own nodes into SBUF staging
            with tc.tile_pool(name=f"psA{hop}", bufs=2, space="PSUM") as psA, \
                 tc.tile_pool(name=f"sbA{hop}", bufs=3) as sbA:
                AU = 2

                def bodyA(t, ui):
                    h_t = h_own[:, ds(t, 1), :].rearrange("p o f -> p (o f)")
                    h_stage = sbA.tile([P, H], F16, tag="hst%d" % ui)
                    nc.vector.tensor_copy(h_stage[:], h_t)
                    trp = psA.tile([H, P], F16, tag="trp%d" % ui)
                    nc.tensor.transpose(trp[:], h_stage[:], id16[:])
                    hT16 = sbA.tile([H, P], F16, tag="hT%d" % ui)
                    nc.scalar.copy(hT16[:], trp[:])
                    knq = psA.tile([P, 2 * H], F32, tag="knq%d" % ui)
                    nc.tensor.matmul(knq[:], lhsT=hT16[:],
                                     rhs=wkq_sb[:, hop, :], start=True,
                                     stop=True)
                    kn_t = kn_stage[:, ds(t, 1), :].rearrange("p o f -> p (o f)")
                    nc.scalar.copy(kn_t, knq[:, 0:H])
                    qb_t = qb_own[:, ds(t, 1), :].rearrange("p o f -> p (o f)")
                    nc.vector.tensor_tensor(out=qb_t, in0=knq[:, H:2 * H],
                                            in1=brep_sb[:, hop, :],
                                            op=mybir.AluOpType.add)

                if not SKIP_A:
                    main = Ttot - (Ttot % AU)
                    if main:
                        with tc.For_i(0, main, AU) as t0:
                            for ui in range(AU):
                                bodyA(t0 + ui, ui)
                    for i, tr in enumerate(range(main, Ttot)):
                        bodyA(tr, i % AU)
                    nc.sync.dma_start(
                        kn_own.ap(),
                        kn_stage[:].rearrange("p t f -> p (t f)"))
                else:
                    nc.vector.memset(qb_own[:], 0)

            # ---- AllGather kn table
            if not SKIP_AG:
                nc.gpsimd.collective_compute(
                    "AllGather", mybir.AluOpType.bypass,
                    replica_groups=[list(range(NCORES))],
                    ins=[kn_own.ap()], outs=[kn_full.ap()])

            # ---- phase C: edge processing
            tile_base = 0
            for j, d in enumerate(tiers_d):
                Tt = tiers_T[j]
                chunks = [CH] * (d // CH) + ([d % CH] if d % CH else [])
                U = meta["tiers_U"][j]
                qb_tier = qb_own[:, tile_base:tile_base + Tt, :]
                h_tier = h_own[:, tile_base:tile_base + Tt, :]
                valid_tier = valid_sb[:, tile_base:tile_base + Tt]
                with tc.tile_pool(name=f"psC{hop}_{j}", bufs=4,
                                  space="PSUM") as psC, \
                     tc.tile_pool(name=f"psP{hop}_{j}", bufs=2,
                                  space="PSUM") as psP, \
                     tc.tile_pool(name=f"sbC{hop}_{j}", bufs=2) as sbC:

                    def bodyC(t, ui, pooledT, d=d, j=j, chunks=chunks,
                              qb_tier=qb_tier, h_tier=h_tier,
                              valid_tier=valid_tier, psC=psC, sbC=sbC):
                        idx_sb = sbC.tile([P, d], I32, tag="idx%d" % ui)
                        nc.sync.dma_start(
                            idx_sb[:],
                            t_in[f"idx{j}"][ds(t, 1)]
                            .rearrange("o p d -> p (o d)"))
                        lgm_sb = sbC.tile([P, d], F16, tag="lgm%d" % ui)
                        nc.sync.dma_start(
                            lgm_sb[:],
                            t_in[f"lgm{j}"][ds(t, 1)]
                            .rearrange("o p d -> p (o d)"))
                        knr = sbC.tile([P, d, H], F16, tag="knr%d" % ui)
                        if SKIP_GATHER:
                            nc.vector.memset(knr[:], 0)
                        else:
                            for s0 in range(d):
                                nc.gpsimd.indirect_dma_start(
                                    out=knr[:, s0, :], out_offset=None,
                                    in_=kn_view,
                                    in_offset=bass.IndirectOffsetOnAxis(
                                        ap=idx_sb[:, s0:s0 + 1], axis=0))
                        edge_sb = sbC.tile([H, d * P], F16, tag="edge%d" % ui)
                        nc.sync.dma_start(
                            edge_sb[:],
                            t_in[f"edge{j}"][ds(t, 1)]
                            .rearrange("o f s -> f (o s)"))
                        qb_t = qb_tier[:, ds(t, 1), :] \
                            .rearrange("p o f -> p (o f)")
                        qbt = sbC.tile([P, H], F16, tag="qb%d" % ui)
                        nc.vector.tensor_copy(qbt[:], qb_t)
                        _q = qbt[:]
                        z16 = sbC.tile([P, d, H], F16, tag="z16%d" % ui)
                        ex_t = sbC.tile([P, d], F16, tag="ex%d" % ui)
                        _a = arep_sb[:, hop, :]
                        for k, c in enumerate(chunks):
                            g0 = k * CH
                            a_b = AP(_a.tensor, _a.offset,
                                     [list(_a.ap[0]), [0, c], list(_a.ap[1])])
                            q_b = AP(_q.tensor, _q.offset,
                                     [list(_q.ap[0]), [0, c], list(_q.ap[1])])
                            zps = psC.tile([P, CH * H], F32, tag="z")
                            for s in range(c):
                                g = g0 + s
                                nc.tensor.matmul(
                                    zps[:, s * H:(s + 1) * H],
                                    lhsT=edge_sb[:, g * P:(g + 1) * P],
                                    rhs=we_sb[:, hop, :],
                                    start=True, stop=True)
                            zc = z16[:, g0:g0 + c, :]
                            # z = ke@We + kn[snd]  (the pooled value v)
                            nc.vector.tensor_tensor(
                                out=zc,
                                in0=zps[:, 0:c * H]
                                .rearrange("p (c h) -> p c h", c=c),
                                in1=knr[:, g0:g0 + c, :],
                                op=mybir.AluOpType.add)
                            w16 = sbC.tile([P, CH, H], F16, tag="w%d" % ui)
                            nc.vector.tensor_tensor(
                                out=w16[:, 0:c, :], in0=zc, in1=q_b,
                                op=mybir.AluOpType.add)
                            u16 = sbC.tile([P, CH, H], F16, tag="u%d" % ui)
                            uc = u16[:, 0:c, :]
                            wc = w16[:, 0:c, :]
                            nc.vector.scalar_tensor_tensor(
                                out=uc, in0=wc, scalar=0.2,
                                in1=wc, op0=mybir.AluOpType.mult,
                                op1=mybir.AluOpType.max)
                            au = sbC.tile([P, CH, H], F16, tag="au%d" % ui)
                            nc.vector.tensor_tensor(
                                out=au[:, 0:c, :], in0=uc, in1=a_b,
                                op=mybir.AluOpType.mult)
                            lgt = sbC.tile([P, CH], F16, tag="lg%d" % ui)
                            with nc.allow_low_precision(reason="O(1) logits"):
                                nc.vector.tensor_reduce(
                                    out=lgt[:, 0:c], in_=au[:, 0:c, :],
                                    axis=mybir.AxisListType.X,
                                    op=mybir.AluOpType.add)
                            lgt2 = sbC.tile([P, CH], F16, tag="lg2%d" % ui)
                            nc.vector.tensor_tensor(
                                out=lgt2[:, 0:c], in0=lgt[:, 0:c],
                                in1=lgm_sb[:, g0:g0 + c],
                                op=mybir.AluOpType.add)
                            nc.scalar.activation(
                                ex_t[:, g0:g0 + c], lgt2[:, 0:c],
                                mybir.ActivationFunctionType.Exp)
                            exz = sbC.tile([P, CH, H], F16, tag="exz%d" % ui)
                            _m = ex_t[:, g0:g0 + c]
                            m_b = AP(_m.tensor, _m.offset,
                                     [list(_m.ap[0]), list(_m.ap[1]), [0, H]])
                            nc.vector.tensor_tensor(
                                out=exz[:, 0:c, :], in0=zc,
                                in1=m_b, op=mybir.AluOpType.mult)
                            for s in range(c):
                                nc.tensor.matmul(
                                    pooledT[:, ui * H:(ui + 1) * H],
                                    lhsT=id16[:], rhs=exz[:, s, :],
                                    start=(k == 0 and s == 0),
                                    stop=(k == len(chunks) - 1 and s == c - 1))
                        den = sbC.tile([P, 1], F32, tag="den%d" % ui)
                        with nc.allow_low_precision(reason="den f32"):
                            nc.vector.tensor_reduce(
                                out=den[:], in_=ex_t[:],
                                axis=mybir.AxisListType.X,
                                op=mybir.AluOpType.add)
                        den2 = sbC.tile([P, 1], F32, tag="den2%d" % ui)
                        nc.vector.tensor_scalar(
                            out=den2[:], in0=den[:], scalar1=EPS,
                            scalar2=None, op0=mybir.AluOpType.add)
                        rden = sbC.tile([P, 1], F32, tag="rden%d" % ui)
                        nc.vector.reciprocal(rden[:], den2[:])
                        t1 = sbC.tile([P, H], F32, tag="t1%d" % ui)
                        nc.vector.tensor_scalar(
                            out=t1[:], in0=pooledT[:, ui * H:(ui + 1) * H],
                            scalar1=rden[:], scalar2=None,
                            op0=mybir.AluOpType.mult)
                        h_t = h_tier[:, ds(t, 1), :] \
                            .rearrange("p o f -> p (o f)")
                        nc.vector.tensor_scalar(
                            out=h_t, in0=t1[:],
                            scalar1=valid_tier[:, ds(t, 1)],
                            scalar2=0.0, op0=mybir.AluOpType.mult,
                            op1=mybir.AluOpType.max)

                    if not SKIP_C:
                        main = Tt - (Tt % U)
                        if main:
                            with tc.For_i(0, main, U) as t0:
                                pooledT = psP.tile([P, U * H], F32,
                                                   tag="pool")
                                for ui in range(U):
                                    bodyC(t0 + ui, ui, pooledT)
                        rem = Tt - main
                        if rem:
                            with tc.For_i(main, Tt, 1) as t0:
                                pooledT = psP.tile([P, U * H], F32,
                                                   tag="poolr")
                                bodyC(t0, 0, pooledT)
                tile_base += Tt

        hsum = const.tile([P, H], F32)
        nc.vector.tensor_reduce(out=hsum[:],
                                in_=h_own[:].rearrange("p t f -> p f t"),
                                axis=mybir.AxisListType.X,
                                op=mybir.AluOpType.add)
        nc.sync.dma_start(t_out, hsum[:])


# --------------------------------------------------------------------- entry
def kernel(node_feats, edge_feats, senders, receivers, Wq, Wk, We, b, att):
    from concourse import bacc
    from concourse.bass_utils import run_bass_kernel_spmd

    node_feats = np.asarray(node_feats)
    meta, in_maps = preprocess(node_feats, edge_feats, senders, receivers,
                               Wq, Wk, We, b, att)
    nc = bacc.Bacc("TRN2", target_bir_lowering=False, debug=False,
                   num_devices=NCORES)
    t_in = {}
    for k, v in in_maps[0].items():
        t_in[k] = nc.dram_tensor(k, list(v.shape), mybir.dt.from_np(v.dtype),
                                 kind="ExternalInput").ap()
    t_out = nc.dram_tensor("out", [P, H], F32, kind="ExternalOutput").ap()
    with tile.TileContext(nc) as tc:
        build_program(tc, t_in, t_out, meta)
    nc.compile()
    res = run_bass_kernel_spmd(nc, in_maps, core_ids=list(range(NCORES)))
    global LAST_EXEC_NS
    LAST_EXEC_NS = getattr(res, "exec_time_ns", None)
    total = np.zeros(H, np.float64)
    for r in res.results:
        total += r["out"].astype(np.float64).sum(axis=0)
    return (total / node_feats.shape[0]).astype(np.float32)
